# revision 8
# baseline (speedup 1.0000x reference)
"""CapsuleRewardHead Trainium2 kernel (8-core data parallel), v2.

Math (per batch row b):
    primary = x @ W + b_lin                    [B, 128]  (128 = 8 caps x 16 dim)
    u_hat[b,o,i,j] = sum_c primary[b,i,c] * out_caps[o,i,c,j]
    3 rounds of dynamic routing over N=32 capsule pairs (o,i), D=16
    out[b] = |squash(s_final)|

Per core (2048 rows = 4 supers of 512 cols = 16 chunks of 128 rows):
  - host packs x as bf16 [128p, 4sp, 32hc, 512b] so each x sub-DMA moves
    1 MiB in per-partition-contiguous 8 KiB runs (HWDGE sync ring, 4
    sub-DMAs per super) -> ~47us HBM floor instead of fp32's ~94us.
  - PE: MM1 accumulates primaryT[feat,512] over 32 h-chunks (bias rides as
    a K=1 matmul vs ones); zero-matmul fillers bridge DMA-chase gaps so the
    PE p-state never drops back to half clock.  MM2 produces u_hat[b,(o,i,j)]
    via block-diag capsule matrices, plus t0 = sum_n u_hat via a summed-caps
    matmul (round 0's uniform-coefficient sum for free).
  - routing per super-batch (K=4 chunks), software-pipelined as
    r0(sp) -> r2(sp-1) -> r1(sp) so ACT exps overlap DVE work of the
    neighbouring round:
      * exp on ACT per chunk with accum_out giving 16*softmax-denominator
        for free; r2 subtracts the row max (logits reach ~76).
      * n- and d-contractions as unit-stride bf16 halving trees on DVE
        (2x dual-pump mode) instead of strided 1x tensor_reduce.
      * t is scaled by alpha *before* the agreement multiply, so the
        b-update is a plain add and alpha folds into one tiny [p,K] op.
      * sqrt via bit-trick seed; the x256 scale of the accum-denominator
        algebra folds into the magic constant (exponent arithmetic).
"""

import os

import numpy as np
import ml_dtypes

B = 16384
HIDDEN = 4096
NUM_OBJ = 4
NUM_CAPS = 8
CAP_DIM = 16
N_ROUTE = 32
N_CORES = 8

LAST_EXEC_TIME_NS = None

BF16 = ml_dtypes.bfloat16
SQRT_MAGIC = 0x1FBD1DF5
SQRT_MAGIC_X256 = SQRT_MAGIC + (16 << 22)  # sqrt(q)*256 via exponent shift

WARM0 = 22   # PE fillers before the first real matmul (DVFS ramp ~15us wall)
WARM_HB = 1  # fillers between h-block groups inside a super
WARM_SP = 2  # fillers at super boundaries
WARM_U = 2   # fillers covering the primt-copy wait before MM2
NQ = 4       # x sub-DMAs per super (1 MiB each)
OFFLOAD = True  # move some big elementwise ops to GPSIMD (non-tail batches)


def _ap(ap, dims):
    import concourse.bass as bass

    return bass.AP(tensor=ap.tensor, offset=ap.offset, ap=dims)


def build_bass(hidden=HIDDEN, b_sh=B // N_CORES):
    import concourse.tile as tile
    from concourse import bacc, mybir

    NH = hidden // 128
    NSP = b_sh // 512
    CPS = 4
    NCH = b_sh // 128
    N, D = N_ROUTE, CAP_DIM
    dt = mybir.dt
    AX = mybir.AxisListType
    OP = mybir.AluOpType
    AF = mybir.ActivationFunctionType
    HQ = NH // NQ

    nc = bacc.Bacc("TRN2", target_bir_lowering=False, debug=False, num_devices=N_CORES)

    xp_ap = nc.dram_tensor(
        "xp", [128, NSP, NH, 512], dt.bfloat16, kind="ExternalInput"
    ).ap()
    w_ap = nc.dram_tensor("w", [128, NH, 128], dt.bfloat16, kind="ExternalInput").ap()
    caps_ap = nc.dram_tensor(
        "caps", [128, NUM_OBJ, 128], dt.bfloat16, kind="ExternalInput"
    ).ap()
    capsum_ap = nc.dram_tensor(
        "capsum", [128, D], dt.bfloat16, kind="ExternalInput"
    ).ap()
    bias_ap = nc.dram_tensor("bias", [1, 384], dt.bfloat16, kind="ExternalInput").ap()
    out_ap = nc.dram_tensor("out", [b_sh], dt.float32, kind="ExternalOutput").ap()
    out_v = out_ap.rearrange("(c p) -> p c", p=128)

    with tile.TileContext(nc) as tc:
        with (
            tc.tile_pool(name="singles", bufs=1) as singles,
            tc.tile_pool(name="xs", bufs=3) as xs_pool,
            tc.tile_pool(name="primt", bufs=2) as primt_pool,
            tc.tile_pool(name="batch", bufs=1) as bpool,
            tc.tile_pool(name="tmp", bufs=2) as tmp_pool,
            tc.tile_pool(name="sm", bufs=4) as sm_pool,
            tc.tile_pool(name="psum_p", bufs=2, space="PSUM") as psp_pool,
            tc.tile_pool(name="psum_u", bufs=4, space="PSUM") as psu_pool,
            tc.tile_pool(name="psum_t", bufs=2, space="PSUM") as pst_pool,
        ):
            # params ride the gpsimd (SWDGE) ring; the sync ring is x-only
            bias_sb = singles.tile([1, 384], dt.bfloat16)
            nc.gpsimd.dma_start(out=bias_sb[:], in_=bias_ap[:, :])
            w_sb = singles.tile([128, NH, 128], dt.bfloat16)
            nc.gpsimd.dma_start(out=w_sb[:], in_=w_ap[:, :, :])
            caps_sb = singles.tile([128, NUM_OBJ, 128], dt.bfloat16)
            nc.gpsimd.dma_start(out=caps_sb[:], in_=caps_ap[:, :, :])
            capsum_sb = singles.tile([128, D], dt.bfloat16)
            nc.gpsimd.dma_start(out=capsum_sb[:], in_=capsum_ap[:, :])
            magic_sb = singles.tile([128, 1], dt.uint32)
            nc.vector.memset(magic_sb[:], SQRT_MAGIC)
            magic2_sb = singles.tile([128, 1], dt.uint32)
            nc.vector.memset(magic2_sb[:], SQRT_MAGIC_X256)
            out_sb = singles.tile([128, NCH], dt.float32)
            # DMA-free zeros so PE warm-up fillers start immediately
            zeros_sb = singles.tile([1, 128], dt.bfloat16)
            nc.vector.memset(zeros_sb[:], 0)

            zl = zeros_sb[:, :]
            zbc = _ap(zl, [zl.ap[0], [0, CPS], [1, 128]])
            ones_l = bias_sb[:, 128:256]
            ones_bc = _ap(ones_l, [ones_l.ap[0], [0, CPS], [1, 128]])
            caps_flat = caps_sb.rearrange("p o f -> p (o f)")

            uh_all, t_all, b_all = {}, {}, {}
            for bi in range(NSP):
                uh_all[bi] = bpool.tile(
                    [128, CPS, N, D], dt.bfloat16, tag=f"uh{bi}", name=f"uh{bi}"
                )
                t_all[bi] = bpool.tile(
                    [128, CPS, D], dt.float32, tag=f"t{bi}", name=f"t{bi}"
                )
                b_all[bi] = bpool.tile(
                    [128, CPS, N], dt.float32, tag=f"b{bi}", name=f"b{bi}"
                )

            def smt(shape, tag, dtype=dt.float32):
                return sm_pool.tile([128, *shape], dtype, tag=tag, name=tag)

            def sqrt_chain(q_ap, magic, K):
                qu = q_ap.bitcast(dt.uint32)
                s1 = smt([K], "sq_sh", dt.uint32)
                nc.vector.tensor_single_scalar(
                    s1[:], qu, 1, op=OP.logical_shift_right
                )
                s2 = smt([K], "sq_sm", dt.uint32)
                nc.vector.tensor_tensor(
                    s2[:], s1[:], _ap(magic[:], [magic[:].ap[0], [0, K]]), op=OP.add
                )
                return s2.bitcast(dt.float32)

            def qchain(bi, se16, magic, K, r):
                """alpha = sqrt(q)/den; r0: den=q+N^2, else den=(se16^2+256q)/256."""
                tt = t_all[bi]
                sq = tmp_pool.tile([128, K, D], dt.float32, tag="sq", name="sq")
                nc.vector.tensor_tensor(sq[:], tt[:], tt[:], op=OP.mult)
                # q = sum_d sq via halving tree (tensor_reduce pays ~150ns
                # per outer row; the tree stays near the 58-cycle op floor)
                qf1 = smt([K, 8], "qf1")
                nc.vector.tensor_tensor(
                    qf1[:], sq[:, :, 0:8], sq[:, :, 8:16], op=OP.add
                )
                qf2 = smt([K, 4], "qf2")
                nc.vector.tensor_tensor(
                    qf2[:], qf1[:, :, 0:4], qf1[:, :, 4:8], op=OP.add
                )
                qf3 = smt([K, 2], "qf3")
                nc.vector.tensor_tensor(
                    qf3[:], qf2[:, :, 0:2], qf2[:, :, 2:4], op=OP.add
                )
                q = smt([K], "q")
                nc.vector.tensor_tensor(
                    q[:], qf3[:, :, 0:1], qf3[:, :, 1:2], op=OP.add
                )
                den = smt([K], "den")
                q256 = None
                if r == 0:
                    nc.vector.tensor_single_scalar(
                        den[:], q[:], float(N * N), op=OP.add
                    )
                else:
                    q256 = smt([K], "q256")
                    nc.vector.tensor_single_scalar(q256[:], q[:], 256.0, op=OP.mult)
                    se2 = smt([K], "se2")
                    nc.vector.tensor_mul(se2[:], se16[:], se16[:])
                    nc.vector.tensor_add(den[:], se2[:], q256[:])
                rden = smt([K], "rden")
                nc.vector.reciprocal(rden[:], den[:])
                if r == 2:
                    return None, q256, rden
                sm = sqrt_chain(q[:], magic, K)
                al = smt([K], "al")
                nc.vector.tensor_mul(al[:], sm, rden[:])
                return al, q256, rden

            def agreement(bi, al, K, first):
                """b += sum_d uh * (t*alpha) via bf16 d-halving tree."""
                tt = t_all[bi]
                tbs = smt([K, D], "tbs", dt.bfloat16)
                nc.vector.tensor_tensor(
                    tbs[:], tt[:], _ap(al[:], [*al[:].ap, [0, D]]), op=OP.mult
                )
                uh = uh_all[bi]
                am = tmp_pool.tile([128, K, N, D], dt.bfloat16, tag="amul", name="amul")
                tb = tbs[:]
                nc.vector.tensor_tensor(
                    am.rearrange("p k n d -> p (k n d)"),
                    uh.rearrange("p k n d -> p (k n d)"),
                    _ap(tb, [tb.ap[0], tb.ap[1], [0, N], tb.ap[2]]),
                    op=OP.mult,
                )
                df1 = tmp_pool.tile([128, K, N, 8], dt.bfloat16, tag="df1", name="df1")
                eng = nc.gpsimd if (OFFLOAD and bi < NSP - 1) else nc.vector
                eng.tensor_tensor(
                    df1[:], am[:, :, :, 0:8], am[:, :, :, 8:16], op=OP.add
                )
                df2 = tmp_pool.tile([128, K, N, 4], dt.bfloat16, tag="df2", name="df2")
                nc.vector.tensor_tensor(
                    df2[:], df1[:, :, :, 0:4], df1[:, :, :, 4:8], op=OP.add
                )
                df3 = tmp_pool.tile([128, K, N, 2], dt.bfloat16, tag="df3", name="df3")
                nc.vector.tensor_tensor(
                    df3[:], df2[:, :, :, 0:2], df2[:, :, :, 2:4], op=OP.add
                )
                if first:
                    nc.vector.tensor_tensor(
                        b_all[bi][:], df3[:, :, :, 0:1], df3[:, :, :, 1:2], op=OP.add
                    )
                else:
                    dta = smt([K, N], "dta")
                    nc.vector.tensor_tensor(
                        dta[:], df3[:, :, :, 0:1], df3[:, :, :, 1:2], op=OP.add
                    )
                    nc.vector.tensor_tensor(
                        b_all[bi][:], b_all[bi][:], dta[:], op=OP.add
                    )

            def exps(bi, src, K):
                """per-chunk ACT exp, bf16, with accum_out = 16*sum_n e."""
                er = tmp_pool.tile([128, K, N, D], dt.bfloat16, tag="erep", name="erep")
                se16 = smt([K], "se16")
                for k in range(K):
                    sk = src[:, k : k + 1, :]
                    nc.scalar.activation(
                        er[:, k : k + 1],
                        _ap(sk, [*sk.ap, [0, D]]),
                        AF.Exp,
                        accum_out=se16[:, k : k + 1],
                    )
                return er, se16

            def tsum(bi, er, K, per_chunk=False, wmul_eng=None, tf1_eng=None):
                """t = sum_n uh*e via bf16 n-halving tree (fp32 final fold)."""
                uh = uh_all[bi]
                wm = tmp_pool.tile([128, K, N, D], dt.bfloat16, tag="wmul", name="wmul")
                tf1 = tmp_pool.tile([128, K, 16, D], dt.bfloat16, tag="tf1", name="tf1")
                tf2 = tmp_pool.tile([128, K, 8, D], dt.bfloat16, tag="tf2", name="tf2")
                tf3 = tmp_pool.tile([128, K, 4, D], dt.bfloat16, tag="tf3", name="tf3")
                tf4 = tmp_pool.tile([128, K, 2, D], dt.bfloat16, tag="tf4", name="tf4")
                tt = t_all[bi]

                def stages(sl):
                    eng = wmul_eng or nc.vector
                    eng.tensor_tensor(
                        wm[:, sl].rearrange("p k n d -> p (k n d)"),
                        uh[:, sl].rearrange("p k n d -> p (k n d)"),
                        er[:, sl].rearrange("p k n d -> p (k n d)"),
                        op=OP.mult,
                    )
                    (tf1_eng or nc.vector).tensor_tensor(
                        tf1[:, sl], wm[:, sl, 0:16, :], wm[:, sl, 16:32, :], op=OP.add
                    )
                    nc.vector.tensor_tensor(
                        tf2[:, sl], tf1[:, sl, 0:8, :], tf1[:, sl, 8:16, :], op=OP.add
                    )
                    nc.vector.tensor_tensor(
                        tf3[:, sl], tf2[:, sl, 0:4, :], tf2[:, sl, 4:8, :], op=OP.add
                    )
                    nc.vector.tensor_tensor(
                        tf4[:, sl], tf3[:, sl, 0:2, :], tf3[:, sl, 2:4, :], op=OP.add
                    )
                    nc.vector.tensor_tensor(
                        tt[:, sl], tf4[:, sl, 0:1, :], tf4[:, sl, 1:2, :], op=OP.add
                    )

                if per_chunk:
                    for k in range(K):
                        stages(slice(k, k + 1))
                else:
                    stages(slice(0, K))

            def routing_r0(bi):
                """round 0 (t0 from PE) + issue r1 exps."""
                K = CPS
                al0, _, _ = qchain(bi, None, magic_sb, K, 0)
                agreement(bi, al0, K, first=True)
                return exps(bi, b_all[bi], K)

            def routing_r1(bi, er1, se16_1):
                """round 1 + prep (mx/bsub) and issue r2 exps."""
                K = CPS
                tail = bi == NSP - 1
                weng = nc.gpsimd if (OFFLOAD and not tail) else None
                tsum(bi, er1, K, per_chunk=tail, wmul_eng=weng)
                al1, _, _ = qchain(bi, se16_1, magic2_sb, K, 1)
                agreement(bi, al1, K, first=False)
                mx = smt([K], "mx")
                nc.vector.tensor_reduce(mx[:], b_all[bi][:], axis=AX.X, op=OP.max)
                bsub = tmp_pool.tile(
                    [128, K, N], dt.float32, tag="bsub", name="bsub"
                )
                nc.vector.tensor_tensor(
                    bsub[:],
                    b_all[bi][:],
                    _ap(mx[:], [*mx[:].ap, [0, N]]),
                    op=OP.subtract,
                )
                return exps(bi, bsub, K)

            def routing_r2(bi, er2, se16_2, per_chunk=False):
                K = CPS
                teng = nc.gpsimd if (OFFLOAD and bi < NSP - 1) else None
                tsum(bi, er2, K, per_chunk=per_chunk, tf1_eng=teng)
                _, q256, rden = qchain(bi, se16_2, magic2_sb, K, 2)
                nc.vector.tensor_mul(
                    out_sb[:, bi * K : (bi + 1) * K], q256[:], rden[:]
                )
                nc.scalar.dma_start(
                    out=out_v[:, bi * K : (bi + 1) * K],
                    in_=out_sb[:, bi * K : (bi + 1) * K],
                )

            def pe_mm(sp):
                xs = xs_pool.tile([128, NH, 512], dt.bfloat16)
                for qd in range(NQ):
                    nc.sync.dma_start(
                        out=xs[:, qd * HQ : (qd + 1) * HQ, :],
                        in_=xp_ap[:, sp, qd * HQ : (qd + 1) * HQ, :],
                    )
                psp = psp_pool.tile([128, 512], dt.float32)
                state = {"first": True}

                def filler(k):
                    for _ in range(k):
                        nc.tensor.matmul(
                            psp[:], zl, zbc, start=state["first"], stop=False
                        )
                        state["first"] = False

                filler(WARM0 if sp == 0 else WARM_SP)
                nc.tensor.matmul(
                    psp[:], bias_sb[:, 0:128], ones_bc, start=False, stop=False
                )
                for qd in range(NQ):
                    for h in range(qd * HQ, (qd + 1) * HQ):
                        nc.tensor.matmul(
                            psp[:],
                            w_sb[:, h, :],
                            xs[:, h, :],
                            start=False,
                            stop=(h == NH - 1),
                        )
                    if qd < NQ - 1:
                        filler(WARM_HB)
                primt = primt_pool.tile([128, 512], dt.bfloat16)
                nc.scalar.copy(primt[:], psp[:])
                for c in range(CPS):
                    lhsT = primt[:, c * 128 : (c + 1) * 128]
                    psu = psu_pool.tile([128, NUM_OBJ * 128], dt.float32)
                    if c == 0 and WARM_U:
                        for wi in range(WARM_U):
                            nc.tensor.matmul(
                                psu[:], zl, zbc, start=(wi == 0), stop=False
                            )
                        nc.tensor.matmul(
                            psu[:], lhsT, caps_flat, start=False, stop=True
                        )
                    else:
                        nc.tensor.matmul(
                            psu[:], lhsT, caps_flat, start=True, stop=True
                        )
                    pst = pst_pool.tile([128, D], dt.float32)
                    nc.tensor.matmul(pst[:], lhsT, capsum_sb[:], start=True, stop=True)
                    nc.scalar.copy(t_all[sp][:, c, :], pst[:])
                    nc.scalar.copy(
                        uh_all[sp][:, c, :, :].rearrange("p n d -> p (n d)"), psu[:]
                    )

            # ---- pipeline: r0(sp) -> r2(sp-1) -> r1(sp) ----
            er_se_r2 = {}
            for sp in range(NSP):
                pe_mm(sp)
                er1, se1 = routing_r0(sp)
                if sp > 0:
                    routing_r2(sp - 1, *er_se_r2[sp - 1])
                er_se_r2[sp] = routing_r1(sp, er1, se1)
            routing_r2(NSP - 1, *er_se_r2[NSP - 1], per_chunk=True)

    nc.compile()
    return nc


def _prep_params(W, b_lin, out_caps, hidden=HIDDEN):
    NH = hidden // 128
    # w[p, hc, f] = W[hc*128+p, f]
    w_f = np.ascontiguousarray(
        W.astype(np.float32).reshape(NH, 128, NUM_CAPS * CAP_DIM).transpose(1, 0, 2)
    ).astype(BF16)
    caps_bd = np.zeros((NUM_OBJ, 128, 128), np.float32)
    for o in range(NUM_OBJ):
        for i in range(NUM_CAPS):
            caps_bd[
                o, i * CAP_DIM : (i + 1) * CAP_DIM, i * CAP_DIM : (i + 1) * CAP_DIM
            ] = out_caps[o, i]
    capsum = caps_bd.sum(0)
    caps_h = np.ascontiguousarray(caps_bd.transpose(1, 0, 2)).astype(BF16)
    capsum_t0 = np.zeros((128, CAP_DIM), np.float32)
    for i in range(NUM_CAPS):
        capsum_t0[i * CAP_DIM : (i + 1) * CAP_DIM, :] = capsum[
            i * CAP_DIM : (i + 1) * CAP_DIM, i * CAP_DIM : (i + 1) * CAP_DIM
        ]
    bias_row = np.concatenate(
        [
            b_lin.astype(np.float32).reshape(1, 128),
            np.ones((1, 128), np.float32),
            np.zeros((1, 128), np.float32),
        ],
        axis=1,
    )
    return (
        w_f,
        caps_h,
        np.ascontiguousarray(capsum_t0).astype(BF16),
        bias_row.astype(BF16),
    )


_NC_CACHE = {}


def kernel(x, W, b_lin, out_caps):
    global LAST_EXEC_TIME_NS
    from concourse.bass_utils import run_bass_kernel_spmd

    x = np.asarray(x)
    W = np.asarray(W)
    b_lin = np.asarray(b_lin)
    out_caps = np.asarray(out_caps)
    bsz, hidden = x.shape
    b_sh = bsz // N_CORES
    nsp = b_sh // 512
    nh = hidden // 128

    key = (hidden, b_sh)
    if key not in _NC_CACHE:
        _NC_CACHE[key] = build_bass(hidden=hidden, b_sh=b_sh)
    nc = _NC_CACHE[key]

    w_f, caps_h, capsum_t0, bias_row = _prep_params(W, b_lin, out_caps, hidden)

    # xp[core][p][sp][hc][b] = x[core*b_sh + sp*512 + b, hc*128 + p], bf16
    xb = x.astype(BF16)
    xp = np.ascontiguousarray(
        xb.reshape(N_CORES, nsp, 512, nh, 128).transpose(0, 4, 1, 3, 2)
    )

    in_maps = []
    for i in range(N_CORES):
        in_maps.append(
            {
                "xp": xp[i],
                "w": w_f,
                "caps": caps_h,
                "capsum": capsum_t0,
                "bias": bias_row,
            }
        )

    res = run_bass_kernel_spmd(
        nc,
        in_maps,
        core_ids=list(range(N_CORES)),
        trace=bool(int(os.environ.get("BASS_TRACE", "0") or "0")),
    )
    LAST_EXEC_TIME_NS = res.exec_time_ns
    return np.concatenate([res.results[i]["out"] for i in range(N_CORES)])


# revision 11
# speedup vs baseline: 1.1676x; 1.1676x over previous
"""CapsuleRewardHead Trainium2 kernel (8-core data parallel), v2.

Math (per batch row b):
    primary = x @ W + b_lin                    [B, 128]  (128 = 8 caps x 16 dim)
    u_hat[b,o,i,j] = sum_c primary[b,i,c] * out_caps[o,i,c,j]
    3 rounds of dynamic routing over N=32 capsule pairs (o,i), D=16
    out[b] = |squash(s_final)|

Per core (2048 rows = 4 supers of 512 cols = 16 chunks of 128 rows):
  - host packs x as bf16 [128p, 4sp, 32hc, 512b] so each x sub-DMA moves
    1 MiB in per-partition-contiguous 8 KiB runs (HWDGE sync ring, 4
    sub-DMAs per super) -> ~47us HBM floor instead of fp32's ~94us.
  - PE: MM1 accumulates primaryT[feat,512] over 32 h-chunks (bias rides as
    a K=1 matmul vs ones); zero-matmul fillers bridge DMA-chase gaps so the
    PE p-state never drops back to half clock.  MM2 produces u_hat[b,(o,i,j)]
    via block-diag capsule matrices, plus t0 = sum_n u_hat via a summed-caps
    matmul (round 0's uniform-coefficient sum for free).
  - routing per super-batch (K=4 chunks), software-pipelined as
    r0(sp) -> r2(sp-1) -> r1(sp) so ACT exps overlap DVE work of the
    neighbouring round:
      * exp on ACT per chunk with accum_out giving 16*softmax-denominator
        for free; r2 subtracts the row max (logits reach ~76).
      * n- and d-contractions as unit-stride bf16 halving trees on DVE
        (2x dual-pump mode) instead of strided 1x tensor_reduce.
      * t is scaled by alpha *before* the agreement multiply, so the
        b-update is a plain add and alpha folds into one tiny [p,K] op.
      * sqrt via bit-trick seed; the x256 scale of the accum-denominator
        algebra folds into the magic constant (exponent arithmetic).
"""

import os

import numpy as np
import ml_dtypes

B = 16384
HIDDEN = 4096
NUM_OBJ = 4
NUM_CAPS = 8
CAP_DIM = 16
N_ROUTE = 32
N_CORES = 8

LAST_EXEC_TIME_NS = None

BF16 = ml_dtypes.bfloat16
SQRT_MAGIC = 0x1FBD1DF5
SQRT_MAGIC_X256 = SQRT_MAGIC + (16 << 22)  # sqrt(q)*256 via exponent shift

WARM0 = 22   # PE fillers before the first real matmul (DVFS ramp ~15us wall)
WARM_HB = 1  # fillers between h-block groups inside a super
WARM_SP = 2  # fillers at super boundaries
WARM_U = 2   # fillers covering the primt-copy wait before MM2
NQ = 4       # x sub-DMAs per super (1 MiB each)
OFFLOAD = False  # GPSIMD TT measured ~0.5 elem/ns — too slow to offload to


def _ap(ap, dims):
    import concourse.bass as bass

    return bass.AP(tensor=ap.tensor, offset=ap.offset, ap=dims)


def build_bass(hidden=HIDDEN, b_sh=B // N_CORES):
    import concourse.tile as tile
    from concourse import bacc, mybir

    NH = hidden // 128
    NSP = b_sh // 512
    CPS = 4
    NCH = b_sh // 128
    N, D = N_ROUTE, CAP_DIM
    dt = mybir.dt
    AX = mybir.AxisListType
    OP = mybir.AluOpType
    AF = mybir.ActivationFunctionType
    HQ = NH // NQ

    nc = bacc.Bacc("TRN2", target_bir_lowering=False, debug=False, num_devices=N_CORES)

    xp_ap = nc.dram_tensor(
        "xp", [128, NSP, NH, 512], dt.bfloat16, kind="ExternalInput"
    ).ap()
    w_ap = nc.dram_tensor("w", [128, NH, 128], dt.bfloat16, kind="ExternalInput").ap()
    caps_ap = nc.dram_tensor(
        "caps", [128, NUM_OBJ, 128], dt.bfloat16, kind="ExternalInput"
    ).ap()
    capsum_ap = nc.dram_tensor(
        "capsum", [128, D], dt.bfloat16, kind="ExternalInput"
    ).ap()
    bias_ap = nc.dram_tensor("bias", [1, 384], dt.bfloat16, kind="ExternalInput").ap()
    out_ap = nc.dram_tensor("out", [b_sh], dt.float32, kind="ExternalOutput").ap()
    out_v = out_ap.rearrange("(c p) -> p c", p=128)

    with tile.TileContext(nc) as tc:
        with (
            tc.tile_pool(name="singles", bufs=1) as singles,
            tc.tile_pool(name="xs", bufs=3) as xs_pool,
            tc.tile_pool(name="primt", bufs=2) as primt_pool,
            tc.tile_pool(name="batch", bufs=1) as bpool,
            tc.tile_pool(name="tmp", bufs=2) as tmp_pool,
            tc.tile_pool(name="sm", bufs=4) as sm_pool,
            tc.tile_pool(name="psum_p", bufs=2, space="PSUM") as psp_pool,
            tc.tile_pool(name="psum_u", bufs=4, space="PSUM") as psu_pool,
            tc.tile_pool(name="psum_t", bufs=2, space="PSUM") as pst_pool,
        ):
            # params ride the gpsimd (SWDGE) ring; the sync ring is x-only
            bias_sb = singles.tile([1, 384], dt.bfloat16)
            nc.gpsimd.dma_start(out=bias_sb[:], in_=bias_ap[:, :])
            w_sb = singles.tile([128, NH, 128], dt.bfloat16)
            nc.gpsimd.dma_start(out=w_sb[:], in_=w_ap[:, :, :])
            caps_sb = singles.tile([128, NUM_OBJ, 128], dt.bfloat16)
            nc.gpsimd.dma_start(out=caps_sb[:], in_=caps_ap[:, :, :])
            capsum_sb = singles.tile([128, D], dt.bfloat16)
            nc.gpsimd.dma_start(out=capsum_sb[:], in_=capsum_ap[:, :])
            magic_sb = singles.tile([128, 1], dt.uint32)
            nc.vector.memset(magic_sb[:], SQRT_MAGIC)
            magic2_sb = singles.tile([128, 1], dt.uint32)
            nc.vector.memset(magic2_sb[:], SQRT_MAGIC_X256)
            out_sb = singles.tile([128, NCH], dt.float32)
            # DMA-free zeros so PE warm-up fillers start immediately
            zeros_sb = singles.tile([1, 128], dt.bfloat16)
            nc.vector.memset(zeros_sb[:], 0)

            zl = zeros_sb[:, :]
            zbc = _ap(zl, [zl.ap[0], [0, CPS], [1, 128]])
            ones_l = bias_sb[:, 128:256]
            ones_bc = _ap(ones_l, [ones_l.ap[0], [0, CPS], [1, 128]])
            caps_flat = caps_sb.rearrange("p o f -> p (o f)")

            uh_all, t_all, b_all = {}, {}, {}
            for bi in range(NSP):
                uh_all[bi] = bpool.tile(
                    [128, CPS, N, D], dt.bfloat16, tag=f"uh{bi}", name=f"uh{bi}"
                )
                t_all[bi] = bpool.tile(
                    [128, CPS, D], dt.float32, tag=f"t{bi}", name=f"t{bi}"
                )
                b_all[bi] = bpool.tile(
                    [128, CPS, N], dt.float32, tag=f"b{bi}", name=f"b{bi}"
                )

            def smt(shape, tag, dtype=dt.float32):
                return sm_pool.tile([128, *shape], dtype, tag=tag, name=tag)

            def sqrt_chain(q_ap, magic, K):
                qu = q_ap.bitcast(dt.uint32)
                s1 = smt([K], "sq_sh", dt.uint32)
                nc.vector.tensor_single_scalar(
                    s1[:], qu, 1, op=OP.logical_shift_right
                )
                s2 = smt([K], "sq_sm", dt.uint32)
                nc.vector.tensor_tensor(
                    s2[:], s1[:], _ap(magic[:], [magic[:].ap[0], [0, K]]), op=OP.add
                )
                return s2.bitcast(dt.float32)

            def qchain(bi, se16, magic, K, r):
                """alpha = sqrt(q)/den; r0: den=q+N^2, else den=(se16^2+256q)/256."""
                tt = t_all[bi]
                sq = tmp_pool.tile([128, K, D], dt.float32, tag="sq", name="sq")
                nc.vector.tensor_tensor(sq[:], tt[:], tt[:], op=OP.mult)
                # q = sum_d sq via halving tree (tensor_reduce pays ~150ns
                # per outer row; the tree stays near the 58-cycle op floor)
                qf1 = smt([K, 8], "qf1")
                nc.vector.tensor_tensor(
                    qf1[:], sq[:, :, 0:8], sq[:, :, 8:16], op=OP.add
                )
                qf2 = smt([K, 4], "qf2")
                nc.vector.tensor_tensor(
                    qf2[:], qf1[:, :, 0:4], qf1[:, :, 4:8], op=OP.add
                )
                qf3 = smt([K, 2], "qf3")
                nc.vector.tensor_tensor(
                    qf3[:], qf2[:, :, 0:2], qf2[:, :, 2:4], op=OP.add
                )
                q = smt([K], "q")
                nc.vector.tensor_tensor(
                    q[:], qf3[:, :, 0:1], qf3[:, :, 1:2], op=OP.add
                )
                den = smt([K], "den")
                q256 = None
                if r == 0:
                    nc.vector.tensor_single_scalar(
                        den[:], q[:], float(N * N), op=OP.add
                    )
                else:
                    q256 = smt([K], "q256")
                    nc.vector.tensor_single_scalar(q256[:], q[:], 256.0, op=OP.mult)
                    se2 = smt([K], "se2")
                    nc.vector.tensor_mul(se2[:], se16[:], se16[:])
                    nc.vector.tensor_add(den[:], se2[:], q256[:])
                rden = smt([K], "rden")
                nc.vector.reciprocal(rden[:], den[:])
                if r == 2:
                    return None, q256, rden
                sm = sqrt_chain(q[:], magic, K)
                al = smt([K], "al")
                nc.vector.tensor_mul(al[:], sm, rden[:])
                return al, q256, rden

            def agreement(bi, al, K, first):
                """b += sum_d uh * (t*alpha) via bf16 d-halving tree."""
                tt = t_all[bi]
                tbs = smt([K, D], "tbs", dt.bfloat16)
                nc.vector.tensor_tensor(
                    tbs[:], tt[:], _ap(al[:], [*al[:].ap, [0, D]]), op=OP.mult
                )
                uh = uh_all[bi]
                am = tmp_pool.tile([128, K, N, D], dt.bfloat16, tag="amul", name="amul")
                tb = tbs[:]
                nc.vector.tensor_tensor(
                    am.rearrange("p k n d -> p (k n d)"),
                    uh.rearrange("p k n d -> p (k n d)"),
                    _ap(tb, [tb.ap[0], tb.ap[1], [0, N], tb.ap[2]]),
                    op=OP.mult,
                )
                df1 = tmp_pool.tile([128, K, N, 8], dt.bfloat16, tag="df1", name="df1")
                eng = nc.gpsimd if (OFFLOAD and bi < NSP - 1) else nc.vector
                eng.tensor_tensor(
                    df1[:], am[:, :, :, 0:8], am[:, :, :, 8:16], op=OP.add
                )
                df2 = tmp_pool.tile([128, K, N, 4], dt.bfloat16, tag="df2", name="df2")
                nc.vector.tensor_tensor(
                    df2[:], df1[:, :, :, 0:4], df1[:, :, :, 4:8], op=OP.add
                )
                df3 = tmp_pool.tile([128, K, N, 2], dt.bfloat16, tag="df3", name="df3")
                nc.vector.tensor_tensor(
                    df3[:], df2[:, :, :, 0:2], df2[:, :, :, 2:4], op=OP.add
                )
                if first:
                    nc.vector.tensor_tensor(
                        b_all[bi][:], df3[:, :, :, 0:1], df3[:, :, :, 1:2], op=OP.add
                    )
                else:
                    dta = smt([K, N], "dta")
                    nc.vector.tensor_tensor(
                        dta[:], df3[:, :, :, 0:1], df3[:, :, :, 1:2], op=OP.add
                    )
                    nc.vector.tensor_tensor(
                        b_all[bi][:], b_all[bi][:], dta[:], op=OP.add
                    )

            def exps(bi, src, K, bias=None):
                """per-chunk ACT exp, bf16, with accum_out = 16*sum_n e.

                bias, if given, is a [p, K] AP whose per-chunk column rides the
                ACT bias port (exp(x + bias_k)) — the r2 row-max subtract."""
                er = tmp_pool.tile([128, K, N, D], dt.bfloat16, tag="erep", name="erep")
                se16 = smt([K], "se16")
                for k in range(K):
                    sk = src[:, k : k + 1, :]
                    kw = {}
                    if bias is not None:
                        kw["bias"] = bias[:, k : k + 1]
                    nc.scalar.activation(
                        er[:, k : k + 1],
                        _ap(sk, [*sk.ap, [0, D]]),
                        AF.Exp,
                        accum_out=se16[:, k : k + 1],
                        **kw,
                    )
                return er, se16

            def tsum(bi, er, K, per_chunk=False, wmul_eng=None, tf1_eng=None):
                """t = sum_n uh*e via bf16 n-halving tree (fp32 final fold)."""
                uh = uh_all[bi]
                wm = tmp_pool.tile([128, K, N, D], dt.bfloat16, tag="wmul", name="wmul")
                tf1 = tmp_pool.tile([128, K, 16, D], dt.bfloat16, tag="tf1", name="tf1")
                tf2 = tmp_pool.tile([128, K, 8, D], dt.bfloat16, tag="tf2", name="tf2")
                tf3 = tmp_pool.tile([128, K, 4, D], dt.bfloat16, tag="tf3", name="tf3")
                tf4 = tmp_pool.tile([128, K, 2, D], dt.bfloat16, tag="tf4", name="tf4")
                tt = t_all[bi]

                def stages(sl):
                    eng = wmul_eng or nc.vector
                    eng.tensor_tensor(
                        wm[:, sl].rearrange("p k n d -> p (k n d)"),
                        uh[:, sl].rearrange("p k n d -> p (k n d)"),
                        er[:, sl].rearrange("p k n d -> p (k n d)"),
                        op=OP.mult,
                    )
                    (tf1_eng or nc.vector).tensor_tensor(
                        tf1[:, sl], wm[:, sl, 0:16, :], wm[:, sl, 16:32, :], op=OP.add
                    )
                    nc.vector.tensor_tensor(
                        tf2[:, sl], tf1[:, sl, 0:8, :], tf1[:, sl, 8:16, :], op=OP.add
                    )
                    nc.vector.tensor_tensor(
                        tf3[:, sl], tf2[:, sl, 0:4, :], tf2[:, sl, 4:8, :], op=OP.add
                    )
                    nc.vector.tensor_tensor(
                        tf4[:, sl], tf3[:, sl, 0:2, :], tf3[:, sl, 2:4, :], op=OP.add
                    )
                    nc.vector.tensor_tensor(
                        tt[:, sl], tf4[:, sl, 0:1, :], tf4[:, sl, 1:2, :], op=OP.add
                    )

                if per_chunk:
                    for k in range(K):
                        stages(slice(k, k + 1))
                else:
                    stages(slice(0, K))

            def routing_r0(bi):
                """round 0 (t0 from PE) + issue r1 exps."""
                K = CPS
                al0, _, _ = qchain(bi, None, magic_sb, K, 0)
                agreement(bi, al0, K, first=True)
                return exps(bi, b_all[bi], K)

            def routing_r1(bi, er1, se16_1):
                """round 1 + prep (mx/bsub) and issue r2 exps."""
                K = CPS
                tail = bi == NSP - 1
                weng = nc.gpsimd if (OFFLOAD and not tail) else None
                tsum(bi, er1, K, per_chunk=tail, wmul_eng=weng)
                al1, _, _ = qchain(bi, se16_1, magic2_sb, K, 1)
                agreement(bi, al1, K, first=False)
                # -rowmax rides the r2 exp's bias port (saves the subtract op)
                mxn = smt([K], "mxn")
                nc.vector.tensor_reduce(
                    mxn[:], b_all[bi][:], axis=AX.X, op=OP.max, negate=True
                )
                return exps(bi, b_all[bi], K, bias=mxn)

            def routing_r2(bi, er2, se16_2, per_chunk=False):
                K = CPS
                teng = nc.gpsimd if (OFFLOAD and bi < NSP - 1) else None
                tsum(bi, er2, K, per_chunk=per_chunk, tf1_eng=teng)
                _, q256, rden = qchain(bi, se16_2, magic2_sb, K, 2)
                nc.vector.tensor_mul(
                    out_sb[:, bi * K : (bi + 1) * K], q256[:], rden[:]
                )
                nc.scalar.dma_start(
                    out=out_v[:, bi * K : (bi + 1) * K],
                    in_=out_sb[:, bi * K : (bi + 1) * K],
                )

            def pe_mm(sp):
                xs = xs_pool.tile([128, NH, 512], dt.bfloat16)
                for qd in range(NQ):
                    nc.sync.dma_start(
                        out=xs[:, qd * HQ : (qd + 1) * HQ, :],
                        in_=xp_ap[:, sp, qd * HQ : (qd + 1) * HQ, :],
                    )
                psp = psp_pool.tile([128, 512], dt.float32)
                state = {"first": True}

                def filler(k):
                    for _ in range(k):
                        nc.tensor.matmul(
                            psp[:], zl, zbc, start=state["first"], stop=False
                        )
                        state["first"] = False

                filler(WARM0 if sp == 0 else WARM_SP)
                nc.tensor.matmul(
                    psp[:], bias_sb[:, 0:128], ones_bc, start=False, stop=False
                )
                for qd in range(NQ):
                    for h in range(qd * HQ, (qd + 1) * HQ):
                        nc.tensor.matmul(
                            psp[:],
                            w_sb[:, h, :],
                            xs[:, h, :],
                            start=False,
                            stop=(h == NH - 1),
                        )
                    if qd < NQ - 1:
                        filler(WARM_HB)
                primt = primt_pool.tile([128, 512], dt.bfloat16)
                nc.scalar.copy(primt[:], psp[:])
                for c in range(CPS):
                    lhsT = primt[:, c * 128 : (c + 1) * 128]
                    psu = psu_pool.tile([128, NUM_OBJ * 128], dt.float32)
                    if c == 0 and WARM_U:
                        for wi in range(WARM_U):
                            nc.tensor.matmul(
                                psu[:], zl, zbc, start=(wi == 0), stop=False
                            )
                        nc.tensor.matmul(
                            psu[:], lhsT, caps_flat, start=False, stop=True
                        )
                    else:
                        nc.tensor.matmul(
                            psu[:], lhsT, caps_flat, start=True, stop=True
                        )
                    pst = pst_pool.tile([128, D], dt.float32)
                    nc.tensor.matmul(pst[:], lhsT, capsum_sb[:], start=True, stop=True)
                    nc.scalar.copy(t_all[sp][:, c, :], pst[:])
                    nc.scalar.copy(
                        uh_all[sp][:, c, :, :].rearrange("p n d -> p (n d)"), psu[:]
                    )

            # ---- pipeline: r0(sp) -> r2(sp-1) -> r1(sp) ----
            er_se_r2 = {}
            for sp in range(NSP):
                pe_mm(sp)
                er1, se1 = routing_r0(sp)
                if sp > 0:
                    routing_r2(sp - 1, *er_se_r2[sp - 1])
                er_se_r2[sp] = routing_r1(sp, er1, se1)
            routing_r2(NSP - 1, *er_se_r2[NSP - 1], per_chunk=True)

    nc.compile()
    return nc


def _prep_params(W, b_lin, out_caps, hidden=HIDDEN):
    NH = hidden // 128
    # w[p, hc, f] = W[hc*128+p, f]
    w_f = np.ascontiguousarray(
        W.astype(np.float32).reshape(NH, 128, NUM_CAPS * CAP_DIM).transpose(1, 0, 2)
    ).astype(BF16)
    caps_bd = np.zeros((NUM_OBJ, 128, 128), np.float32)
    for o in range(NUM_OBJ):
        for i in range(NUM_CAPS):
            caps_bd[
                o, i * CAP_DIM : (i + 1) * CAP_DIM, i * CAP_DIM : (i + 1) * CAP_DIM
            ] = out_caps[o, i]
    capsum = caps_bd.sum(0)
    caps_h = np.ascontiguousarray(caps_bd.transpose(1, 0, 2)).astype(BF16)
    capsum_t0 = np.zeros((128, CAP_DIM), np.float32)
    for i in range(NUM_CAPS):
        capsum_t0[i * CAP_DIM : (i + 1) * CAP_DIM, :] = capsum[
            i * CAP_DIM : (i + 1) * CAP_DIM, i * CAP_DIM : (i + 1) * CAP_DIM
        ]
    bias_row = np.concatenate(
        [
            b_lin.astype(np.float32).reshape(1, 128),
            np.ones((1, 128), np.float32),
            np.zeros((1, 128), np.float32),
        ],
        axis=1,
    )
    return (
        w_f,
        caps_h,
        np.ascontiguousarray(capsum_t0).astype(BF16),
        bias_row.astype(BF16),
    )


_NC_CACHE = {}


def kernel(x, W, b_lin, out_caps):
    global LAST_EXEC_TIME_NS
    from concourse.bass_utils import run_bass_kernel_spmd

    x = np.asarray(x)
    W = np.asarray(W)
    b_lin = np.asarray(b_lin)
    out_caps = np.asarray(out_caps)
    bsz, hidden = x.shape
    b_sh = bsz // N_CORES
    nsp = b_sh // 512
    nh = hidden // 128

    key = (hidden, b_sh)
    if key not in _NC_CACHE:
        _NC_CACHE[key] = build_bass(hidden=hidden, b_sh=b_sh)
    nc = _NC_CACHE[key]

    w_f, caps_h, capsum_t0, bias_row = _prep_params(W, b_lin, out_caps, hidden)

    # xp[core][p][sp][hc][b] = x[core*b_sh + sp*512 + b, hc*128 + p], bf16
    xb = x.astype(BF16)
    xp = np.ascontiguousarray(
        xb.reshape(N_CORES, nsp, 512, nh, 128).transpose(0, 4, 1, 3, 2)
    )

    in_maps = []
    for i in range(N_CORES):
        in_maps.append(
            {
                "xp": xp[i],
                "w": w_f,
                "caps": caps_h,
                "capsum": capsum_t0,
                "bias": bias_row,
            }
        )

    res = run_bass_kernel_spmd(
        nc,
        in_maps,
        core_ids=list(range(N_CORES)),
        trace=bool(int(os.environ.get("BASS_TRACE", "0") or "0")),
    )
    LAST_EXEC_TIME_NS = res.exec_time_ns
    return np.concatenate([res.results[i]["out"] for i in range(N_CORES)])
